# revision 11
# baseline (speedup 1.0000x reference)
"""Trainium2 Bass kernel for nn_MessagePassingLayer (gnn_message_passing).

Computes, for x:[B,C,N,1] f32, edge_index:[B,N,K] i32, alpha scalar:
    out[b,c,n] = x[b,c,n]*(1+alpha) + sum_k x[b,c,edge_index[b,n,k]]

Sharding: B=8 batch samples, one per NeuronCore (data parallel). Edge
indices are intra-sample so there is no cross-core communication.

Per-core device program:
  - load node-major table xt [N=4096, C=64] (host-transposed layout)
  - SWDGE dma_gather rounds (4 queues) fetching 256B rows from HBM; the
    Q7 descriptor-generation rate (~994ns + 7.5ns/idx per gather, serial
    per queue-pair) is the wall, so chunk sizes taper so the final
    round's DMA burst + accumulate + store tail is short
  - DVE running accumulation per chunk; out = xt*(1+alpha) + sum
  - stores split across both HWDGE engines; host transposes back
"""
import os
import sys
import types

import numpy as np

B, C, N, K = 8, 64, 4096, 16
NCORES = 8
P = 128
COLS = N // P  # 32 nodes per partition
FREE = COLS * C  # 2048 f32 per partition

# Gather schedule: list of (segment_len, k_per_gather). Segment lens sum
# to N. Each gather instruction covers k_per_gather neighbor-slots of one
# node segment (num_idxs = seg_len * kpg), so early segments use big
# gathers (amortize the ~1us fixed SWDGE cost per instruction) and late
# segments small ones (short final drain). seg_len % 128 == 0, K % kpg == 0,
# and K // kpg should be a multiple of 4 for queue balance.
SCHED = [tuple(int(v) for v in part.split("x"))
         for part in os.environ.get(
             "KERNEL_SCHED", "2048x2,1024x4,512x4,384x4,128x4").split(",")]
assert sum(s for s, _ in SCHED) == N
assert all(s % 128 == 0 and K % g == 0 and (K // g) % 4 == 0
           for s, g in SCHED)

SCRATCH = int(os.environ.get("KERNEL_SCRATCH", "16384"))
SINGLE_PACKET = bool(int(os.environ.get("KERNEL_SINGLE_PACKET", "0")))
INTERLEAVE = bool(int(os.environ.get("KERNEL_INTERLEAVE", "0")))

LAST_EXEC_NS = None


# ---------------------------------------------------------------------------
# axon NTFF profile hook shim (the agent image's antenv lacks axon_hooks)
# ---------------------------------------------------------------------------
def _install_profile_shim():
    if "antenv.axon_hooks" in sys.modules:
        return
    try:
        import antenv

        mod = types.ModuleType("antenv.axon_hooks")
        mod._hook = None
        mod.set_axon_ntff_profile_hook = lambda h: setattr(mod, "_hook", h)
        mod.get_axon_ntff_profile_hook = lambda: mod._hook
        sys.modules["antenv.axon_hooks"] = mod
        antenv.axon_hooks = mod
        from trn_agent_boot.trn_boot import _ntff_profile_via_ctypes

        mod.set_axon_ntff_profile_hook(
            _ntff_profile_via_ctypes("/opt/axon/libaxon_pjrt.so")
        )
    except Exception:
        pass


# ---------------------------------------------------------------------------
# Walrus in this container rejects >1 sync-wait per instruction. Split any
# multi-wait instruction into single-wait NoOps on the same engine.
# ---------------------------------------------------------------------------
def _split_multiwaits(nc, mybir):
    cnt = [0]
    for f in nc.m.functions:
        for bb in f.blocks:
            new_list = []
            for ins in bb.instructions:
                si = ins.sync_info
                if si is not None and si.on_wait and len(si.on_wait) > 1:
                    waits = list(si.on_wait)
                    for w in waits[:-1]:
                        cnt[0] += 1
                        nop = mybir.InstNoOp(name=f"I-waitsplit-{cnt[0]}")
                        nop.engine = ins.engine
                        nop.sync_info = mybir.SyncInfo(on_wait=[w], on_update=[])
                        try:
                            nc.register_instruction(nop, overwrite=True)
                        except Exception:
                            pass
                        new_list.append(nop)
                    ins.sync_info = mybir.SyncInfo(
                        on_wait=[waits[-1]], on_update=list(si.on_update)
                    )
                new_list.append(ins)
            bb.instructions = new_list


# ---------------------------------------------------------------------------
# Device program
# ---------------------------------------------------------------------------
def _build_program():
    import concourse.mybir as mybir
    import concourse.tile as tile
    from concourse import bacc

    nc = bacc.Bacc("TRN2", target_bir_lowering=False, debug=False,
                   num_devices=NCORES, num_swdge_queues=4,
                   dynamic_dma_scratch_size=SCRATCH)
    xt_d = nc.dram_tensor("xt", [N, C], mybir.dt.float32, kind="ExternalInput")
    idx_d = nc.dram_tensor("idx", [P, K * (N // 16)], mybir.dt.int16,
                           kind="ExternalInput")
    alpha_d = nc.dram_tensor("alpha", [P, 1], mybir.dt.float32,
                             kind="ExternalInput")
    out_d = nc.dram_tensor("out", [N, C], mybir.dt.float32,
                           kind="ExternalOutput")

    nseg = len(SCHED)
    seglen = [s for s, _ in SCHED]
    gpk = [g for _, g in SCHED]            # k-pieces per gather
    ngat = [K // g for g in gpk]           # gathers per segment
    gsz = [seglen[s] * gpk[s] for s in range(nseg)]   # num_idxs per gather
    ipg = [n // 16 for n in seglen]        # idx cols per k-piece
    segw = [(n // P) * C for n in seglen]  # f32 cols per segment
    ioff = [sum(K * i for i in ipg[:s]) for s in range(nseg + 1)]
    soff = [sum(segw[:s]) for s in range(nseg + 1)]

    with tile.TileContext(nc) as tc:
        with tc.tile_pool(name="sbuf", bufs=1) as pool:
            xt_sb = pool.tile([P, FREE], mybir.dt.float32, tag="xt")
            idx_sb = [pool.tile([P, K * ipg[s]], mybir.dt.int16,
                                tag=f"idx{s}", name=f"idx{s}")
                      for s in range(nseg)]
            al_sb = pool.tile([P, 1], mybir.dt.float32, tag="al")
            g = [[pool.tile([P, gpk[s] * segw[s]], mybir.dt.float32,
                            tag=f"g{s}_{j}", name=f"g{s}_{j}")
                  for j in range(ngat[s])] for s in range(nseg)]
            o = [pool.tile([P, segw[s]], mybir.dt.float32, tag=f"o{s}",
                           name=f"o{s}") for s in range(nseg)]
            xt_nm = xt_d.ap().rearrange("(p a) c -> p (a c)", p=P)
            out_nm = out_d.ap().rearrange("(p a) c -> p (a c)", p=P)

            # segment-0 idx load first, split across both HWDGE engines so
            # the first gathers start ASAP; later segments' idx tiles are
            # loaded after the first gather wave is issued.
            engs = [nc.sync, nc.scalar]
            h0 = K * ipg[0] // 2
            for j in range(2):
                engs[j].dma_start(
                    out=idx_sb[0][:, j * h0:(j + 1) * h0],
                    in_=idx_d.ap()[:, j * h0:(j + 1) * h0],
                )
            # per-gather-size index-count registers
            nregs = {}
            for n in sorted(set(gsz)):
                nregs[n] = nc.gpsimd.to_reg(n)

            gi = 0
            for s in range(nseg):
                for j in range(ngat[s]):
                    nc.gpsimd.dma_gather(
                        out_ap=g[s][j][:].rearrange("p (a c) -> p a c", c=C),
                        in_ap=xt_d.ap(),
                        idxs_ap=idx_sb[s][:, j * gpk[s] * ipg[s]:
                                          (j + 1) * gpk[s] * ipg[s]],
                        num_idxs=gsz[s],
                        num_idxs_reg=nregs[gsz[s]],
                        elem_size=C,
                        queue_num=gi % 4,
                        single_packet=SINGLE_PACKET,
                    )
                    gi += 1
                    if gi == 4:
                        # remaining idx segments + xt/alpha, issued after
                        # the first gather wave so they never gate gather 0
                        for ss in range(1, nseg):
                            nc.sync.dma_start(
                                out=idx_sb[ss][:],
                                in_=idx_d.ap()[:, ioff[ss]:ioff[ss + 1]],
                            )
                        nc.scalar.dma_start(out=al_sb[:], in_=alpha_d.ap())
                        nc.scalar.dma_start(out=xt_sb[:], in_=xt_nm)
                        nc.scalar.add(out=al_sb[:], in_=al_sb[:], add=1.0)

            for s in range(nseg):
                # o = xt*(1+alpha), then running accumulation in gather
                # completion (issue) order so only the last add is tail
                nc.vector.tensor_scalar_mul(
                    out=o[s][:], in0=xt_sb[:, soff[s]:soff[s + 1]],
                    scalar1=al_sb[:, :1],
                )
                for j in range(ngat[s]):
                    for t in range(gpk[s]):
                        nc.vector.tensor_add(
                            out=o[s][:], in0=o[s][:],
                            in1=g[s][j][:, t * segw[s]:(t + 1) * segw[s]],
                        )
                # split the store across both HWDGE engines so the tail
                # store after the final add drains in half the time
                h = segw[s] // 2
                for j in range(2):
                    engs[j].dma_start(
                        out=out_nm[:, soff[s] + j * h:soff[s] + (j + 1) * h],
                        in_=o[s][:, j * h:(j + 1) * h],
                    )

    nc.compile()
    _split_multiwaits(nc, mybir)
    return nc


_PROGRAM = None


def _get_program():
    global _PROGRAM
    if _PROGRAM is None:
        _PROGRAM = _build_program()
    return _PROGRAM


# ---------------------------------------------------------------------------
# Host glue
# ---------------------------------------------------------------------------
_slot = np.arange(N)
_PERM = (_slot % P) * COLS + (_slot // P)  # node id for flat gather slot i


def _prep_idx(edge_b):
    """edge_b [N, K] int32 -> wrapped int16 [128, K*N/16] for dma_gather,
    laid out segment-major, k-major within a segment; a merged gather for
    (segment s, gather j) reads the contiguous column band covering its
    k_per_gather k-pieces."""
    ids = edge_b[_PERM, :].astype(np.int16)          # [4096 slots, K]
    f = ids.T.reshape(K, N // 16, 16)                # [K, s=256, p16]
    w = np.transpose(f, (2, 0, 1))                   # [p16, K, 256]
    w = np.tile(w, (8, 1, 1))                        # [128, K, 256]
    parts = []
    off = 0
    for n, _ in SCHED:
        lo, hi = off // 16, (off + n) // 16
        parts.append(w[:, :, lo:hi].reshape(P, K * (hi - lo)))
        off += n
    return np.ascontiguousarray(np.concatenate(parts, axis=1))


def kernel(x, edge_index, alpha):
    global LAST_EXEC_NS
    _install_profile_shim()
    from concourse import bass_utils

    x = np.asarray(x)
    edge_index = np.asarray(edge_index)
    alpha_v = np.float32(np.asarray(alpha))

    nc = _get_program()

    xt = np.transpose(x[..., 0], (0, 2, 1))  # [B, N, C]
    in_maps = []
    for b in range(B):
        in_maps.append({
            "xt": np.ascontiguousarray(xt[b]),
            "idx": _prep_idx(edge_index[b]),
            "alpha": np.full((P, 1), alpha_v, dtype=np.float32),
        })

    trace = bool(int(os.environ.get("KERNEL_PROFILE", "0")))
    res = bass_utils.run_bass_kernel_spmd(
        nc, in_maps, core_ids=list(range(NCORES)), trace=trace
    )
    LAST_EXEC_NS = res.exec_time_ns

    out = np.empty((B, C, N, 1), dtype=np.float32)
    for b in range(B):
        out[b, :, :, 0] = res.results[b]["out"].T
    return out


# revision 13
# speedup vs baseline: 1.2044x; 1.2044x over previous
"""Trainium2 Bass kernel for nn_MessagePassingLayer (gnn_message_passing).

Computes, for x:[B,C,N,1] f32, edge_index:[B,N,K] i32, alpha scalar:
    out[b,c,n] = x[b,c,n]*(1+alpha) + sum_k x[b,c,edge_index[b,n,k]]

Sharding: B=8 batch samples, one per NeuronCore (data parallel). Edge
indices are intra-sample so there is no cross-core communication.

Per-core device program:
  - load node-major table xt [N=4096, C=64] (host-transposed layout)
  - SWDGE dma_gather rounds (4 queues) fetching 256B rows from HBM; the
    Q7 descriptor-generation rate (~994ns + 7.5ns/idx per gather, serial
    per queue-pair) is the wall, so chunk sizes taper so the final
    round's DMA burst + accumulate + store tail is short
  - DVE running accumulation per chunk; out = xt*(1+alpha) + sum
  - stores split across both HWDGE engines; host transposes back
"""
import os
import sys
import types

import numpy as np

B, C, N, K = 8, 64, 4096, 16
NCORES = 8
P = 128
COLS = N // P  # 32 nodes per partition
FREE = COLS * C  # 2048 f32 per partition

# Gather schedule: list of (segment_len, k_per_gather). Segment lens sum
# to N; each gather covers k_per_gather neighbor-slots of one node segment
# (num_idxs = seg_len * kpg). Measured Q7 desc-gen rate is non-monotonic
# in gather size (7.5ns/idx at 2048, degrading to 9.5-12ns/idx at
# 3584-4096), so 2048-idx gathers with a tapered tail (short final DMA
# drain) are optimal. seg_len % 128 == 0, K % kpg == 0, (K//kpg) % 4 == 0.
SCHED = [tuple(int(v) for v in part.split("x"))
         for part in os.environ.get(
             "KERNEL_SCHED", "2048x1,1536x1,512x1").split(",")]
assert sum(s for s, _ in SCHED) == N
assert all(s % 128 == 0 and K % g == 0 and (K // g) % 4 == 0
           for s, g in SCHED)

SCRATCH = int(os.environ.get("KERNEL_SCRATCH", "16384"))
SINGLE_PACKET = bool(int(os.environ.get("KERNEL_SINGLE_PACKET", "0")))
INTERLEAVE = bool(int(os.environ.get("KERNEL_INTERLEAVE", "0")))

LAST_EXEC_NS = None


# ---------------------------------------------------------------------------
# axon NTFF profile hook shim (the agent image's antenv lacks axon_hooks)
# ---------------------------------------------------------------------------
def _install_profile_shim():
    if "antenv.axon_hooks" in sys.modules:
        return
    try:
        import antenv

        mod = types.ModuleType("antenv.axon_hooks")
        mod._hook = None
        mod.set_axon_ntff_profile_hook = lambda h: setattr(mod, "_hook", h)
        mod.get_axon_ntff_profile_hook = lambda: mod._hook
        sys.modules["antenv.axon_hooks"] = mod
        antenv.axon_hooks = mod
        from trn_agent_boot.trn_boot import _ntff_profile_via_ctypes

        mod.set_axon_ntff_profile_hook(
            _ntff_profile_via_ctypes("/opt/axon/libaxon_pjrt.so")
        )
    except Exception:
        pass


# ---------------------------------------------------------------------------
# Walrus in this container rejects >1 sync-wait per instruction. Split any
# multi-wait instruction into single-wait NoOps on the same engine.
# ---------------------------------------------------------------------------
def _split_multiwaits(nc, mybir):
    cnt = [0]
    for f in nc.m.functions:
        for bb in f.blocks:
            new_list = []
            for ins in bb.instructions:
                si = ins.sync_info
                if si is not None and si.on_wait and len(si.on_wait) > 1:
                    waits = list(si.on_wait)
                    for w in waits[:-1]:
                        cnt[0] += 1
                        nop = mybir.InstNoOp(name=f"I-waitsplit-{cnt[0]}")
                        nop.engine = ins.engine
                        nop.sync_info = mybir.SyncInfo(on_wait=[w], on_update=[])
                        try:
                            nc.register_instruction(nop, overwrite=True)
                        except Exception:
                            pass
                        new_list.append(nop)
                    ins.sync_info = mybir.SyncInfo(
                        on_wait=[waits[-1]], on_update=list(si.on_update)
                    )
                new_list.append(ins)
            bb.instructions = new_list


# ---------------------------------------------------------------------------
# Device program
# ---------------------------------------------------------------------------
def _build_program():
    import concourse.mybir as mybir
    import concourse.tile as tile
    from concourse import bacc

    nc = bacc.Bacc("TRN2", target_bir_lowering=False, debug=False,
                   num_devices=NCORES, num_swdge_queues=4,
                   dynamic_dma_scratch_size=SCRATCH)
    xt_d = nc.dram_tensor("xt", [N, C], mybir.dt.float32, kind="ExternalInput")
    idx_d = nc.dram_tensor("idx", [P, K * (N // 16)], mybir.dt.int16,
                           kind="ExternalInput")
    alpha_d = nc.dram_tensor("alpha", [P, 1], mybir.dt.float32,
                             kind="ExternalInput")
    out_d = nc.dram_tensor("out", [N, C], mybir.dt.float32,
                           kind="ExternalOutput")

    nseg = len(SCHED)
    seglen = [s for s, _ in SCHED]
    gpk = [g for _, g in SCHED]            # k-pieces per gather
    ngat = [K // g for g in gpk]           # gathers per segment
    gsz = [seglen[s] * gpk[s] for s in range(nseg)]   # num_idxs per gather
    ipg = [n // 16 for n in seglen]        # idx cols per k-piece
    segw = [(n // P) * C for n in seglen]  # f32 cols per segment
    ioff = [sum(K * i for i in ipg[:s]) for s in range(nseg + 1)]
    soff = [sum(segw[:s]) for s in range(nseg + 1)]

    with tile.TileContext(nc) as tc:
        with tc.tile_pool(name="sbuf", bufs=1) as pool:
            xt_sb = pool.tile([P, FREE], mybir.dt.float32, tag="xt")
            idx_sb = [pool.tile([P, K * ipg[s]], mybir.dt.int16,
                                tag=f"idx{s}", name=f"idx{s}")
                      for s in range(nseg)]
            al_sb = pool.tile([P, 1], mybir.dt.float32, tag="al")
            g = [[pool.tile([P, gpk[s] * segw[s]], mybir.dt.float32,
                            tag=f"g{s}_{j}", name=f"g{s}_{j}")
                  for j in range(ngat[s])] for s in range(nseg)]
            o = [pool.tile([P, segw[s]], mybir.dt.float32, tag=f"o{s}",
                           name=f"o{s}") for s in range(nseg)]
            xt_nm = xt_d.ap().rearrange("(p a) c -> p (a c)", p=P)
            out_nm = out_d.ap().rearrange("(p a) c -> p (a c)", p=P)

            # segment-0 idx load first, split across both HWDGE engines so
            # the first gathers start ASAP; later segments' idx tiles are
            # loaded after the first gather wave is issued.
            engs = [nc.sync, nc.scalar]
            h0 = K * ipg[0] // 2
            for j in range(2):
                engs[j].dma_start(
                    out=idx_sb[0][:, j * h0:(j + 1) * h0],
                    in_=idx_d.ap()[:, j * h0:(j + 1) * h0],
                )
            # per-gather-size index-count registers
            nregs = {}
            for n in sorted(set(gsz)):
                nregs[n] = nc.gpsimd.to_reg(n)

            gi = 0
            for s in range(nseg):
                for j in range(ngat[s]):
                    nc.gpsimd.dma_gather(
                        out_ap=g[s][j][:].rearrange("p (a c) -> p a c", c=C),
                        in_ap=xt_d.ap(),
                        idxs_ap=idx_sb[s][:, j * gpk[s] * ipg[s]:
                                          (j + 1) * gpk[s] * ipg[s]],
                        num_idxs=gsz[s],
                        num_idxs_reg=nregs[gsz[s]],
                        elem_size=C,
                        queue_num=gi % 4,
                        single_packet=SINGLE_PACKET,
                    )
                    gi += 1
                    if gi == 4:
                        # remaining idx segments + xt/alpha, issued after
                        # the first gather wave so they never gate gather 0
                        for ss in range(1, nseg):
                            nc.sync.dma_start(
                                out=idx_sb[ss][:],
                                in_=idx_d.ap()[:, ioff[ss]:ioff[ss + 1]],
                            )
                        nc.scalar.dma_start(out=al_sb[:], in_=alpha_d.ap())
                        nc.scalar.dma_start(out=xt_sb[:], in_=xt_nm)
                        nc.scalar.add(out=al_sb[:], in_=al_sb[:], add=1.0)

            for s in range(nseg):
                # o = xt*(1+alpha), then running accumulation in gather
                # completion (issue) order so only the last add is tail
                nc.vector.tensor_scalar_mul(
                    out=o[s][:], in0=xt_sb[:, soff[s]:soff[s + 1]],
                    scalar1=al_sb[:, :1],
                )
                for j in range(ngat[s]):
                    for t in range(gpk[s]):
                        nc.vector.tensor_add(
                            out=o[s][:], in0=o[s][:],
                            in1=g[s][j][:, t * segw[s]:(t + 1) * segw[s]],
                        )
                # split the store across both HWDGE engines so the tail
                # store after the final add drains in half the time
                h = segw[s] // 2
                for j in range(2):
                    engs[j].dma_start(
                        out=out_nm[:, soff[s] + j * h:soff[s] + (j + 1) * h],
                        in_=o[s][:, j * h:(j + 1) * h],
                    )

    nc.compile()
    _split_multiwaits(nc, mybir)
    return nc


_PROGRAM = None


def _get_program():
    global _PROGRAM
    if _PROGRAM is None:
        _PROGRAM = _build_program()
    return _PROGRAM


# ---------------------------------------------------------------------------
# Host glue
# ---------------------------------------------------------------------------
_slot = np.arange(N)
_PERM = (_slot % P) * COLS + (_slot // P)  # node id for flat gather slot i


def _prep_idx(edge_b):
    """edge_b [N, K] int32 -> wrapped int16 [128, K*N/16] for dma_gather,
    laid out segment-major, k-major within a segment; a merged gather for
    (segment s, gather j) reads the contiguous column band covering its
    k_per_gather k-pieces."""
    ids = edge_b[_PERM, :].astype(np.int16)          # [4096 slots, K]
    f = ids.T.reshape(K, N // 16, 16)                # [K, s=256, p16]
    w = np.transpose(f, (2, 0, 1))                   # [p16, K, 256]
    w = np.tile(w, (8, 1, 1))                        # [128, K, 256]
    parts = []
    off = 0
    for n, _ in SCHED:
        lo, hi = off // 16, (off + n) // 16
        parts.append(w[:, :, lo:hi].reshape(P, K * (hi - lo)))
        off += n
    return np.ascontiguousarray(np.concatenate(parts, axis=1))


def kernel(x, edge_index, alpha):
    global LAST_EXEC_NS
    _install_profile_shim()
    from concourse import bass_utils

    x = np.asarray(x)
    edge_index = np.asarray(edge_index)
    alpha_v = np.float32(np.asarray(alpha))

    nc = _get_program()

    xt = np.transpose(x[..., 0], (0, 2, 1))  # [B, N, C]
    in_maps = []
    for b in range(B):
        in_maps.append({
            "xt": np.ascontiguousarray(xt[b]),
            "idx": _prep_idx(edge_index[b]),
            "alpha": np.full((P, 1), alpha_v, dtype=np.float32),
        })

    trace = bool(int(os.environ.get("KERNEL_PROFILE", "0")))
    res = bass_utils.run_bass_kernel_spmd(
        nc, in_maps, core_ids=list(range(NCORES)), trace=trace
    )
    LAST_EXEC_NS = res.exec_time_ns

    out = np.empty((B, C, N, 1), dtype=np.float32)
    for b in range(B):
        out[b, :, :, 0] = res.results[b]["out"].T
    return out


# revision 16
# speedup vs baseline: 1.2218x; 1.0144x over previous
"""Trainium2 Bass kernel for nn_MessagePassingLayer (gnn_message_passing).

Computes, for x:[B,C,N,1] f32, edge_index:[B,N,K] i32, alpha scalar:
    out[b,c,n] = x[b,c,n]*(1+alpha) + sum_k x[b,c,edge_index[b,n,k]]

Sharding: B=8 batch samples, one per NeuronCore (data parallel). Edge
indices are intra-sample so there is no cross-core communication.

Per-core device program:
  - load node-major table xt [N=4096, C=64] (host-transposed layout)
  - SWDGE dma_gather rounds (4 queues) fetching 256B rows from HBM; the
    Q7 descriptor-generation rate (~994ns + 7.5ns/idx per gather, serial
    per queue-pair) is the wall, so chunk sizes taper so the final
    round's DMA burst + accumulate + store tail is short
  - DVE running accumulation per chunk; out = xt*(1+alpha) + sum
  - stores split across both HWDGE engines; host transposes back
"""
import os
import sys
import types

import numpy as np

B, C, N, K = 8, 64, 4096, 16
NCORES = 8
P = 128
COLS = N // P  # 32 nodes per partition
FREE = COLS * C  # 2048 f32 per partition

# Gather schedule: list of (segment_len, k_per_gather). Segment lens sum
# to N; each gather covers k_per_gather neighbor-slots of one node segment
# (num_idxs = seg_len * kpg). Measured Q7 desc-gen rate is non-monotonic
# in gather size (7.5ns/idx at 2048, degrading to 9.5-12ns/idx at
# 3584-4096), so 2048-idx gathers with a tapered tail (short final DMA
# drain) are optimal. seg_len % 128 == 0, K % kpg == 0, (K//kpg) % 4 == 0.
SCHED = [tuple(int(v) for v in part.split("x"))
         for part in os.environ.get(
             "KERNEL_SCHED", "2048x1,1536x1,512x1").split(",")]
assert sum(s for s, _ in SCHED) == N
assert all(s % 128 == 0 and K % g == 0 and (K // g) % 4 == 0
           for s, g in SCHED)

SCRATCH = int(os.environ.get("KERNEL_SCRATCH", "16384"))
SINGLE_PACKET = bool(int(os.environ.get("KERNEL_SINGLE_PACKET", "0")))

LAST_EXEC_NS = None


# ---------------------------------------------------------------------------
# axon NTFF profile hook shim (the agent image's antenv lacks axon_hooks)
# ---------------------------------------------------------------------------
def _install_profile_shim():
    if "antenv.axon_hooks" in sys.modules:
        return
    try:
        import antenv

        mod = types.ModuleType("antenv.axon_hooks")
        mod._hook = None
        mod.set_axon_ntff_profile_hook = lambda h: setattr(mod, "_hook", h)
        mod.get_axon_ntff_profile_hook = lambda: mod._hook
        sys.modules["antenv.axon_hooks"] = mod
        antenv.axon_hooks = mod
        from trn_agent_boot.trn_boot import _ntff_profile_via_ctypes

        mod.set_axon_ntff_profile_hook(
            _ntff_profile_via_ctypes("/opt/axon/libaxon_pjrt.so")
        )
    except Exception:
        pass


# ---------------------------------------------------------------------------
# Walrus in this container rejects >1 sync-wait per instruction. Split any
# multi-wait instruction into single-wait NoOps on the same engine.
# ---------------------------------------------------------------------------
def _split_multiwaits(nc, mybir):
    cnt = [0]
    for f in nc.m.functions:
        for bb in f.blocks:
            new_list = []
            for ins in bb.instructions:
                si = ins.sync_info
                if si is not None and si.on_wait and len(si.on_wait) > 1:
                    waits = list(si.on_wait)
                    for w in waits[:-1]:
                        cnt[0] += 1
                        nop = mybir.InstNoOp(name=f"I-waitsplit-{cnt[0]}")
                        nop.engine = ins.engine
                        nop.sync_info = mybir.SyncInfo(on_wait=[w], on_update=[])
                        try:
                            nc.register_instruction(nop, overwrite=True)
                        except Exception:
                            pass
                        new_list.append(nop)
                    ins.sync_info = mybir.SyncInfo(
                        on_wait=[waits[-1]], on_update=list(si.on_update)
                    )
                new_list.append(ins)
            bb.instructions = new_list


# ---------------------------------------------------------------------------
# Device program
# ---------------------------------------------------------------------------
def _build_program():
    import concourse.mybir as mybir
    import concourse.tile as tile
    from concourse import bacc

    nc = bacc.Bacc("TRN2", target_bir_lowering=False, debug=False,
                   num_devices=NCORES, num_swdge_queues=4,
                   dynamic_dma_scratch_size=SCRATCH)
    xt_d = nc.dram_tensor("xt", [N, C], mybir.dt.float32, kind="ExternalInput")
    idx_d = nc.dram_tensor("idx", [P, K * (N // 16)], mybir.dt.int16,
                           kind="ExternalInput")
    alpha_d = nc.dram_tensor("alpha", [P, 1], mybir.dt.float32,
                             kind="ExternalInput")
    out_d = nc.dram_tensor("out", [N, C], mybir.dt.float32,
                           kind="ExternalOutput")

    nseg = len(SCHED)
    seglen = [s for s, _ in SCHED]
    gpk = [g for _, g in SCHED]            # k-pieces per gather
    ngat = [K // g for g in gpk]           # gathers per segment
    gsz = [seglen[s] * gpk[s] for s in range(nseg)]   # num_idxs per gather
    ipg = [n // 16 for n in seglen]        # idx cols per k-piece
    segw = [(n // P) * C for n in seglen]  # f32 cols per segment
    ioff = [sum(K * i for i in ipg[:s]) for s in range(nseg + 1)]
    soff = [sum(segw[:s]) for s in range(nseg + 1)]

    with tile.TileContext(nc) as tc:
        with tc.tile_pool(name="sbuf", bufs=1) as pool:
            xt_sb = pool.tile([P, FREE], mybir.dt.float32, tag="xt")
            idx_sb = [pool.tile([P, K * ipg[s]], mybir.dt.int16,
                                tag=f"idx{s}", name=f"idx{s}")
                      for s in range(nseg)]
            al_sb = pool.tile([P, 1], mybir.dt.float32, tag="al")
            g = [[pool.tile([P, gpk[s] * segw[s]], mybir.dt.float32,
                            tag=f"g{s}_{j}", name=f"g{s}_{j}")
                  for j in range(ngat[s])] for s in range(nseg)]
            o = [pool.tile([P, segw[s]], mybir.dt.float32, tag=f"o{s}",
                           name=f"o{s}") for s in range(nseg)]
            xt_nm = xt_d.ap().rearrange("(p a) c -> p (a c)", p=P)
            out_nm = out_d.ap().rearrange("(p a) c -> p (a c)", p=P)

            # segment-0 idx load first, split across both HWDGE engines so
            # the first gathers start ASAP; later segments' idx tiles are
            # loaded after the first gather wave is issued.
            engs = [nc.sync, nc.scalar]
            h0 = K * ipg[0] // 2
            for j in range(2):
                engs[j].dma_start(
                    out=idx_sb[0][:, j * h0:(j + 1) * h0],
                    in_=idx_d.ap()[:, j * h0:(j + 1) * h0],
                )
            # per-gather-size index-count registers
            nregs = {}
            for n in sorted(set(gsz)):
                nregs[n] = nc.gpsimd.to_reg(n)

            gi = 0
            for s in range(nseg):
                for j in range(ngat[s]):
                    nc.gpsimd.dma_gather(
                        out_ap=g[s][j][:].rearrange("p (a c) -> p a c", c=C),
                        in_ap=xt_d.ap(),
                        idxs_ap=idx_sb[s][:, j * gpk[s] * ipg[s]:
                                          (j + 1) * gpk[s] * ipg[s]],
                        num_idxs=gsz[s],
                        num_idxs_reg=nregs[gsz[s]],
                        elem_size=C,
                        queue_num=gi % 4,
                        single_packet=SINGLE_PACKET,
                    )
                    gi += 1
                    if gi == 4:
                        # remaining idx segments + xt/alpha, issued after
                        # the first gather wave so they never gate gather 0
                        for ss in range(1, nseg):
                            nc.sync.dma_start(
                                out=idx_sb[ss][:],
                                in_=idx_d.ap()[:, ioff[ss]:ioff[ss + 1]],
                            )
                        nc.scalar.dma_start(out=al_sb[:], in_=alpha_d.ap())
                        nc.scalar.dma_start(out=xt_sb[:], in_=xt_nm)

            for s in range(nseg):
                # o = xt*(1+alpha), then running accumulation in gather
                # completion (issue) order so only the last add is tail
                nc.vector.tensor_scalar_mul(
                    out=o[s][:], in0=xt_sb[:, soff[s]:soff[s + 1]],
                    scalar1=al_sb[:, :1],
                )
                for j in range(ngat[s]):
                    for t in range(gpk[s]):
                        nc.vector.tensor_add(
                            out=o[s][:], in0=o[s][:],
                            in1=g[s][j][:, t * segw[s]:(t + 1) * segw[s]],
                        )
                # split the store across both HWDGE engines so the tail
                # store after the final add drains in half the time
                h = segw[s] // 2
                for j in range(2):
                    engs[j].dma_start(
                        out=out_nm[:, soff[s] + j * h:soff[s] + (j + 1) * h],
                        in_=o[s][:, j * h:(j + 1) * h],
                    )

    nc.compile()
    _split_multiwaits(nc, mybir)
    return nc


_PROGRAM = None


def _get_program():
    global _PROGRAM
    if _PROGRAM is None:
        _PROGRAM = _build_program()
    return _PROGRAM


# ---------------------------------------------------------------------------
# Host glue
# ---------------------------------------------------------------------------
_slot = np.arange(N)
_PERM = (_slot % P) * COLS + (_slot // P)  # node id for flat gather slot i


def _prep_idx(edge_b):
    """edge_b [N, K] int32 -> wrapped int16 [128, K*N/16] for dma_gather,
    laid out segment-major, k-major within a segment; a merged gather for
    (segment s, gather j) reads the contiguous column band covering its
    k_per_gather k-pieces."""
    ids = edge_b[_PERM, :].astype(np.int16)          # [4096 slots, K]
    f = ids.T.reshape(K, N // 16, 16)                # [K, s=256, p16]
    w = np.transpose(f, (2, 0, 1))                   # [p16, K, 256]
    w = np.tile(w, (8, 1, 1))                        # [128, K, 256]
    parts = []
    off = 0
    for n, _ in SCHED:
        lo, hi = off // 16, (off + n) // 16
        parts.append(w[:, :, lo:hi].reshape(P, K * (hi - lo)))
        off += n
    return np.ascontiguousarray(np.concatenate(parts, axis=1))


def kernel(x, edge_index, alpha):
    global LAST_EXEC_NS
    _install_profile_shim()
    from concourse import bass_utils

    x = np.asarray(x)
    edge_index = np.asarray(edge_index)
    alpha_v = np.float32(np.asarray(alpha))

    nc = _get_program()

    xt = np.transpose(x[..., 0], (0, 2, 1))  # [B, N, C]
    in_maps = []
    for b in range(B):
        in_maps.append({
            "xt": np.ascontiguousarray(xt[b]),
            "idx": _prep_idx(edge_index[b]),
            # host pre-adds the +1 so no device-side scalar add is needed
            "alpha": np.full((P, 1), alpha_v + 1.0, dtype=np.float32),
        })

    trace = bool(int(os.environ.get("KERNEL_PROFILE", "0")))
    res = bass_utils.run_bass_kernel_spmd(
        nc, in_maps, core_ids=list(range(NCORES)), trace=trace
    )
    LAST_EXEC_NS = res.exec_time_ns

    out = np.empty((B, C, N, 1), dtype=np.float32)
    for b in range(B):
        out[b, :, :, 0] = res.results[b]["out"].T
    return out
